# revision 3
# baseline (speedup 1.0000x reference)
"""ARD-RBF covariance kernel for Trainium2 (Bass/Tile), 8-core row-parallel.

Math (matches the reference):
    s  = exp(-weights[:, 0])                      # (D,) inverse lengthscales
    Us = U * s ; Vs = V * s
    sq[i, j] = ||Us_i||^2 + ||Vs_j||^2 - 2 Us_i . Vs_j
    K[i, j]  = exp(2*sn) * exp(-0.5 * max(sq, 0))

Device strategy (per core, rows sharded 8 ways):
    One augmented matmul computes sq directly in PSUM:
      lhsT (K=18 x 128) rows: [-2*Us^T block ; ||Us||^2 row ; ones row]
      rhs  (K=18 x 512) rows: [ Vs^T         ; ones row     ; ||Vs||^2 row]
    Then a single ScalarE activation per 2048-wide tile computes
      out = Exp(-0.5 * psum + 2*sn)   (PSUM -> SBUF, fp16), and DMA stores.

The augmented operands (O(N*D) work) are built on the host; the device
does only the O(N*M) work: one DMA-in, the matmul stream, the exp stream,
and the store stream.

Perf notes:
  - Output is written as fp16 (abs err ~5e-4 << 2e-2 tolerance) and widened
    to f32 on the host: halves HBM store traffic to 16 MB/core (~45 us at
    358 GB/s/core).
  - Store DMAs alternate between the qSP HWDGE ring (nc.sync) and the SWDGE
    ring (nc.gpsimd) so per-DMA completion-receipt stalls on one ring
    overlap the data phase of the other.
  - Steady state is bound by ScalarE exp throughput: 8.39M elem/core at
    1 elem/lane/cycle @ 1.2 GHz + 172 cyc/instr init ~= 59 us. PE (fp32
    matmul, 4 concurrent row-group tiles) and stores overlap under it.
"""

import numpy as np

import concourse.bacc as bacc
import concourse.bass as bass  # noqa: F401  (AP helpers)
import concourse.mybir as mybir
import concourse.tile as tile

N, M, D = 8192, 8192, 16
N_CORES = 8
ROWS = N // N_CORES  # 1024 rows of U per core
P = 128              # output partitions per row block
FREE = 512           # matmul moving free dim (one PSUM bank of f32)
QUAD = 2048          # ACT chunk: 4 banks
K = D + 2            # augmented contraction dim

F32 = mybir.dt.float32
F16 = mybir.dt.float16
AF = mybir.ActivationFunctionType


def build_program(rows=ROWS, m_cols=M, repeats=1):
    """Build the per-core Bass program. rows/m_cols shrinkable for sim."""
    rb = rows // P
    nq = m_cols // QUAD

    nc = bacc.Bacc()
    l18 = nc.declare_dram_parameter("l18", [K, rows], F32, isOutput=False)
    r18 = nc.declare_dram_parameter("r18", [K, m_cols], F32, isOutput=False)
    sn2 = nc.declare_dram_parameter("sn2", [1, 1], F32, isOutput=False)
    out = nc.declare_dram_parameter("out", [rows, m_cols], F16, isOutput=True)

    with tile.TileContext(nc) as tc:
        with (
            tc.tile_pool(name="singles", bufs=1) as singles,
            tc.tile_pool(name="psum_pool", bufs=2, space="PSUM") as psum_pool,
            tc.tile_pool(name="obuf_pool", bufs=4) as obuf_pool,
        ):
            # ACT bias: 2*sn broadcast across partitions
            bias2 = singles.tile([P, 1], F32)
            nc.gpsimd.dma_start(bias2[:], sn2[:].to_broadcast((P, 1)))

            # L/R carry 4 copies of the K=18 operand at partitions 0/32/64/96
            # so 4 matmuls can run concurrently in the PE's four 32-row
            # groups (tile_position row tiling) — hides the fp32 LDWEIGHTS
            # and 2-pass matmul cost behind concurrent streaming.
            L = singles.tile([3 * 32 + K, rows], F32)
            nc.sync.dma_start(L[0:K, :], l18[:])
            for g in range(1, 4):
                nc.sync.dma_start(L[32 * g : 32 * g + K, :], L[0:K, :])

            R = singles.tile([3 * 32 + K, m_cols], F32)
            nc.sync.dma_start(R[0:K, :], r18[:])
            for g in range(1, 4):
                nc.sync.dma_start(R[32 * g : 32 * g + K, :], R[0:K, :])

            # --- main loop ----------------------------------------------
            for _rep in range(repeats):
                for m in range(rb):
                    ob = obuf_pool.tile([P, m_cols], F16, tag="ob", name="ob")
                    for q in range(nq):
                        ps = psum_pool.tile([P, QUAD], F32, tag="ps", name="ps")
                        for k in range(QUAD // FREE):
                            n = q * (QUAD // FREE) + k
                            nc.tensor.matmul(
                                ps[:, k * FREE : (k + 1) * FREE],
                                L[32 * k : 32 * k + K, m * P : (m + 1) * P],
                                R[32 * k : 32 * k + K, n * FREE : (n + 1) * FREE],
                                start=True, stop=True,
                                tile_position=(32 * k, 0),
                            )
                        nc.scalar.activation(
                            ob[:, q * QUAD : (q + 1) * QUAD], ps[:],
                            AF.Exp, bias=bias2[:], scale=-0.5,
                        )
                        # store each quad as soon as its ACT lands; alternate
                        # between the qSP HWDGE ring and the SWDGE (gpsimd)
                        # ring so per-DMA completion stalls overlap across
                        # rings instead of serializing on one.
                        eng = nc.sync if (m * nq + q) % 2 == 0 else nc.gpsimd
                        eng.dma_start(
                            out[m * P : (m + 1) * P, q * QUAD : (q + 1) * QUAD],
                            ob[:, q * QUAD : (q + 1) * QUAD],
                        )

    nc.compile()  # bacc lowering: splits multi-waits, reg alloc, etc.
    return nc


_PROGRAM_CACHE = {}


def get_program(rows=ROWS, m_cols=M, repeats=1):
    key = (rows, m_cols, repeats)
    if key not in _PROGRAM_CACHE:
        _PROGRAM_CACHE[key] = build_program(rows, m_cols, repeats)
    return _PROGRAM_CACHE[key]


def make_in_maps(U, V, weights, sn):
    """Host-side O(N*D) prep: scaled/augmented matmul operands per core."""
    U = np.asarray(U, dtype=np.float32)
    V = np.asarray(V, dtype=np.float32)
    w = np.asarray(weights, dtype=np.float32).reshape(D)
    s = np.exp(-w)                                        # (D,)
    us = U * s                                            # (N, D)
    vs = V * s                                            # (M, D)
    u2 = np.einsum("nd,nd->n", us, us, dtype=np.float32)  # (N,)
    v2 = np.einsum("md,md->m", vs, vs, dtype=np.float32)  # (M,)

    r18 = np.empty((K, M), dtype=np.float32)
    r18[0:D] = vs.T
    r18[D] = 1.0
    r18[D + 1] = v2
    r18 = np.ascontiguousarray(r18)

    sn2 = (2.0 * np.asarray(sn, dtype=np.float32)).reshape(1, 1)

    in_maps = []
    for c in range(N_CORES):
        sl = slice(c * ROWS, (c + 1) * ROWS)
        l18 = np.empty((K, ROWS), dtype=np.float32)
        l18[0:D] = -2.0 * us[sl].T
        l18[D] = u2[sl]
        l18[D + 1] = 1.0
        in_maps.append({"l18": np.ascontiguousarray(l18), "r18": r18, "sn2": sn2})
    return in_maps


def kernel(U, V, weights, sn):
    from concourse.bass_utils import run_bass_kernel_spmd

    nc = get_program()
    in_maps = make_in_maps(U, V, weights, sn)
    res = run_bass_kernel_spmd(nc, in_maps, core_ids=list(range(N_CORES)))
    return np.concatenate(
        [np.asarray(r["out"]).astype(np.float32) for r in res.results], axis=0
    )


# revision 5
# speedup vs baseline: 1.0543x; 1.0543x over previous
"""ARD-RBF covariance kernel for Trainium2 (Bass/Tile), 8-core row-parallel.

Math (matches the reference):
    s  = exp(-weights[:, 0])                      # (D,) inverse lengthscales
    Us = U * s ; Vs = V * s
    sq[i, j] = ||Us_i||^2 + ||Vs_j||^2 - 2 Us_i . Vs_j
    K[i, j]  = exp(2*sn) * exp(-0.5 * max(sq, 0))

Device strategy (per core, rows sharded 8 ways):
    One augmented matmul computes sq directly in PSUM:
      lhsT (K=18 x 128) rows: [-2*Us^T block ; ||Us||^2 row ; ones row]
      rhs  (K=18 x 512) rows: [ Vs^T         ; ones row     ; ||Vs||^2 row]
    Then a single ScalarE activation per 2048-wide tile computes
      out = Exp(-0.5 * psum + 2*sn)   (PSUM -> SBUF, fp16), and DMA stores.

The augmented operands (O(N*D) work) are built on the host; the device
does only the O(N*M) work: one DMA-in, the matmul stream, the exp stream,
and the store stream.

Perf notes:
  - Output is written as fp16 (abs err ~5e-4 << 2e-2 tolerance) and widened
    to f32 on the host: halves HBM store traffic to 16 MB/core (~45 us at
    358 GB/s/core).
  - Store DMAs alternate between the qSP HWDGE ring (nc.sync) and the SWDGE
    ring (nc.gpsimd) so per-DMA completion-receipt stalls on one ring
    overlap the data phase of the other.
  - Steady state is bound by ScalarE exp throughput: 8.39M elem/core at
    1 elem/lane/cycle @ 1.2 GHz + 172 cyc/instr init ~= 59 us. PE (fp32
    matmul, 4 concurrent row-group tiles) and stores overlap under it.
"""

import numpy as np

import concourse.bacc as bacc
import concourse.bass as bass  # noqa: F401  (AP helpers)
import concourse.mybir as mybir
import concourse.tile as tile

N, M, D = 8192, 8192, 16
N_CORES = 8
ROWS = N // N_CORES  # 1024 rows of U per core
P = 128              # output partitions per row block
FREE = 512           # matmul moving free dim (one PSUM bank of f32)
QUAD = 2048          # ACT chunk: 4 banks
K = D + 2            # augmented contraction dim

F32 = mybir.dt.float32
F16 = mybir.dt.float16
AF = mybir.ActivationFunctionType


def build_program(rows=ROWS, m_cols=M, repeats=1):
    """Build the per-core Bass program. rows/m_cols shrinkable for sim."""
    rb = rows // P
    nq = m_cols // QUAD

    nc = bacc.Bacc()
    l18 = nc.declare_dram_parameter("l18", [K, rows], F32, isOutput=False)
    r18 = nc.declare_dram_parameter("r18", [K, m_cols], F32, isOutput=False)
    sn2 = nc.declare_dram_parameter("sn2", [P, 1], F32, isOutput=False)
    out = nc.declare_dram_parameter("out", [rows, m_cols], F16, isOutput=True)

    with tile.TileContext(nc) as tc:
        with (
            tc.tile_pool(name="singles", bufs=1) as singles,
            tc.tile_pool(name="psum_pool", bufs=2, space="PSUM") as psum_pool,
            tc.tile_pool(name="obuf_pool", bufs=4) as obuf_pool,
        ):
            # ACT bias: 2*sn, pre-broadcast across partitions on the host
            bias2 = singles.tile([P, 1], F32)
            nc.sync.dma_start(bias2[:], sn2[:])

            # L/R carry 4 copies of the K=18 operand at partitions 0/32/64/96
            # so 4 matmuls can run concurrently in the PE's four 32-row
            # groups (tile_position row tiling) — hides the fp32 LDWEIGHTS
            # and 2-pass matmul cost behind concurrent streaming. The four
            # copies are independent DRAM loads spread over both DMA rings
            # so they pipeline instead of chaining off one SBUF copy.
            L = singles.tile([3 * 32 + K, rows], F32)
            R = singles.tile([3 * 32 + K, m_cols], F32)
            for g in range(4):
                eng = nc.sync if g % 2 == 0 else nc.gpsimd
                eng.dma_start(L[32 * g : 32 * g + K, :], l18[:])
                eng.dma_start(R[32 * g : 32 * g + K, :], r18[:])

            # --- main loop ----------------------------------------------
            for _rep in range(repeats):
                for m in range(rb):
                    ob = obuf_pool.tile([P, m_cols], F16, tag="ob", name="ob")
                    for q in range(nq):
                        ps = psum_pool.tile([P, QUAD], F32, tag="ps", name="ps")
                        for k in range(QUAD // FREE):
                            n = q * (QUAD // FREE) + k
                            nc.tensor.matmul(
                                ps[:, k * FREE : (k + 1) * FREE],
                                L[32 * k : 32 * k + K, m * P : (m + 1) * P],
                                R[32 * k : 32 * k + K, n * FREE : (n + 1) * FREE],
                                start=True, stop=True,
                                tile_position=(32 * k, 0),
                            )
                        nc.scalar.activation(
                            ob[:, q * QUAD : (q + 1) * QUAD], ps[:],
                            AF.Exp, bias=bias2[:], scale=-0.5,
                        )
                        # store each quad as soon as its ACT lands; alternate
                        # between the qSP HWDGE ring and the SWDGE (gpsimd)
                        # ring so per-DMA completion stalls overlap across
                        # rings instead of serializing on one.
                        eng = nc.sync if (m * nq + q) % 2 == 0 else nc.gpsimd
                        eng.dma_start(
                            out[m * P : (m + 1) * P, q * QUAD : (q + 1) * QUAD],
                            ob[:, q * QUAD : (q + 1) * QUAD],
                        )

    nc.compile()  # bacc lowering: splits multi-waits, reg alloc, etc.
    return nc


_PROGRAM_CACHE = {}


def get_program(rows=ROWS, m_cols=M, repeats=1):
    key = (rows, m_cols, repeats)
    if key not in _PROGRAM_CACHE:
        _PROGRAM_CACHE[key] = build_program(rows, m_cols, repeats)
    return _PROGRAM_CACHE[key]


def make_in_maps(U, V, weights, sn):
    """Host-side O(N*D) prep: scaled/augmented matmul operands per core."""
    U = np.asarray(U, dtype=np.float32)
    V = np.asarray(V, dtype=np.float32)
    w = np.asarray(weights, dtype=np.float32).reshape(D)
    s = np.exp(-w)                                        # (D,)
    us = U * s                                            # (N, D)
    vs = V * s                                            # (M, D)
    u2 = np.einsum("nd,nd->n", us, us, dtype=np.float32)  # (N,)
    v2 = np.einsum("md,md->m", vs, vs, dtype=np.float32)  # (M,)

    r18 = np.empty((K, M), dtype=np.float32)
    r18[0:D] = vs.T
    r18[D] = 1.0
    r18[D + 1] = v2
    r18 = np.ascontiguousarray(r18)

    sn2 = np.full((P, 1), 2.0 * float(np.asarray(sn)), dtype=np.float32)

    in_maps = []
    for c in range(N_CORES):
        sl = slice(c * ROWS, (c + 1) * ROWS)
        l18 = np.empty((K, ROWS), dtype=np.float32)
        l18[0:D] = -2.0 * us[sl].T
        l18[D] = u2[sl]
        l18[D + 1] = 1.0
        in_maps.append({"l18": np.ascontiguousarray(l18), "r18": r18, "sn2": sn2})
    return in_maps


def kernel(U, V, weights, sn):
    from concourse.bass_utils import run_bass_kernel_spmd

    nc = get_program()
    in_maps = make_in_maps(U, V, weights, sn)
    res = run_bass_kernel_spmd(nc, in_maps, core_ids=list(range(N_CORES)))
    return np.concatenate(
        [np.asarray(r["out"]).astype(np.float32) for r in res.results], axis=0
    )


# revision 6
# speedup vs baseline: 1.1780x; 1.1173x over previous
"""ARD-RBF covariance kernel for Trainium2 (Bass/Tile), 8-core row-parallel.

Math (matches the reference):
    s  = exp(-weights[:, 0])                      # (D,) inverse lengthscales
    Us = U * s ; Vs = V * s
    sq[i, j] = ||Us_i||^2 + ||Vs_j||^2 - 2 Us_i . Vs_j
    K[i, j]  = exp(2*sn) * exp(-0.5 * max(sq, 0))

Device strategy (per core, rows sharded 8 ways):
    One augmented matmul computes sq directly in PSUM:
      lhsT (K=18 x 128) rows: [-2*Us^T block ; ||Us||^2 - 4*sn ; ones row]
      rhs  (K=18 x 512) rows: [ Vs^T         ; ones row        ; ||Vs||^2 ]
    (the exp(2*sn) factor is folded into the u2 row on the host, so the
    device computes exp(-0.5 * psum) with no bias anywhere.)

    The elementwise exp is the bottleneck (ScalarE is the only table-exp
    engine, 1 elem/lane/cycle @ 1.2 GHz = ~59 us/core alone), so the work
    is SPLIT between two engines per 2048-column superchunk:
      - ScalarE: exp on columns [0,1536)   -> (1536+172)/1.2 = 1.42 us
      - VectorE: columns [1536,2048) via a custom fused 2-instruction
        polynomial (registered at import into dve_ops.OPS):
          op1: m = (1 + c1 t + c2 t^2 + c3 t^3)^2, t = min(sq, 16)
          op2: out = ((m^2)^2)^2        == q(t)^16 ~= exp(-t/2)
        max abs err 3.4e-4 (vs 2.4e-3 budget) -> 2x(512+151)/0.96 = 1.38 us
    Each engine has its own PSUM double-buffer (ACT 2x[128,1536] = 6 banks,
    DVE 2x[128,512] = 2 banks), so they never block each other.

Perf notes:
  - fp16 output (abs err ~1e-4 << tolerance), widened on host: 16 MB/core
    of HBM stores ~= 45 us at 358 GB/s.
  - Store DMAs alternate between the qSP HWDGE ring (nc.sync) and the
    SWDGE ring (nc.gpsimd) so per-DMA completion stalls overlap.
  - Steady state ~= max(ACT 45.5, DVE 44.2, DMA ~46, PE ~20) us.
"""

import numpy as np

import concourse.bacc as bacc
import concourse.bass as bass  # noqa: F401  (AP helpers)
import concourse.mybir as mybir
import concourse.tile as tile

N, M, D = 8192, 8192, 16
N_CORES = 8
ROWS = N // N_CORES  # 1024 rows of U per core
P = 128              # output partitions per row block
FREE = 512           # matmul moving free dim (one PSUM bank of f32)
SUPER = 2048         # superchunk: 3 ACT banks + 1 DVE bank of columns
ACT_W = 1536         # ScalarE exp columns per superchunk
DVE_W = 512          # VectorE poly-exp columns per superchunk
K = D + 2            # augmented contraction dim

F32 = mybir.dt.float32
F16 = mybir.dt.float16
AF = mybir.ActivationFunctionType

# Fitted so that (1 + c1 t + c2 t^2 + c3 t^3)^16 ~= exp(-t/2) on [0, 16];
# clamped beyond (exp(-8) = 3.3e-4 = the max abs error, vs 2.4e-3 budget).
EXP_CLAMP = 16.0
EXP_C1 = -0.031241876640111712
EXP_C2 = 0.00048318304875227515
EXP_C3 = -4.259047924211225e-06

_OP1_NAME = "RBF_EXPQ_ANT"
_OP2_NAME = "RBF_POW8_ANT"
_DVE_OPS = {}


def _register_dve_ops():
    """Register the two fused ops in dve_ops.OPS (documented authoring
    surface; done at import so kernel.py stays self-contained). Idempotent."""
    if _DVE_OPS:
        return _DVE_OPS
    import concourse.dve_ops as dve_ops
    from concourse.dve_ops import DveOp
    from concourse.dve_spec import (
        C0, C1, C2, C3, One, Spec, Src0, _spill_c3_to_src1, minn, sq,
        _has_src1, lower,
    )
    from concourse.dve_uop import DveOpSpec

    existing = {op.name: op for op in dve_ops.OPS}

    def _ref_op1(in0, in1, c0, c1, c2):
        t = np.minimum(in0.astype(np.float32), np.float32(c0))
        c3 = np.float32(EXP_C3)
        q = (((c3 * t + np.float32(c2)) * t + np.float32(c1)) * t
             + np.float32(1.0)).astype(np.float32)
        return (q * q).astype(np.float32)

    def _ref_op2(in0, in1, c0, c1, c2):
        x = in0.astype(np.float32)
        return (((x * x) ** 2) ** 2).astype(np.float32)

    _t = minn(Src0, C0)
    _q = ((C3 * _t + C2) * _t + C1) * _t + One
    specs = {
        _OP1_NAME: Spec(body=_spill_c3_to_src1(sq(_q)), reference=_ref_op1),
        _OP2_NAME: Spec(body=sq(sq(sq(Src0))), reference=_ref_op2),
    }
    for name, spec in specs.items():
        if name in existing:
            _DVE_OPS[name] = existing[name]
            continue
        row = max(dve_ops._SUB_OPCODE_FOR_NAME.values()) + 1
        assert row < 0x20, "no free custom-DVE opcode rows"
        dve_ops._SUB_OPCODE_FOR_NAME[name] = row
        # pin uops_sha to this process's lower() output (self-consistent)
        shas = {}
        for ver in ("v3", "v4"):
            tmp = DveOpSpec(name=name, opcode=row, uops=lower(spec, ver=ver),
                            rd1_en=_has_src1(spec))
            shas[ver] = tmp.sha(ver)
        op = DveOp(name, spec, subdim=False, uops_sha=shas)
        dve_ops.OPS.append(op)
        dve_ops.CUSTOM_DVE_SPECS[name] = spec
        _DVE_OPS[name] = op
    return _DVE_OPS


def build_program(rows=ROWS, m_cols=M, repeats=1):
    """Build the per-core Bass program. rows/m_cols shrinkable for sim."""
    ops = _register_dve_ops()
    op1, op2 = ops[_OP1_NAME], ops[_OP2_NAME]
    rb = rows // P
    nsc = m_cols // SUPER

    nc = bacc.Bacc()
    l18 = nc.declare_dram_parameter("l18", [K, rows], F32, isOutput=False)
    r18 = nc.declare_dram_parameter("r18", [K, m_cols], F32, isOutput=False)
    out = nc.declare_dram_parameter("out", [rows, m_cols], F16, isOutput=True)

    with tile.TileContext(nc) as tc:
        with (
            tc.tile_pool(name="singles", bufs=1) as singles,
            tc.tile_pool(name="apsum", bufs=2, space="PSUM") as apsum,
            tc.tile_pool(name="dpsum", bufs=2, space="PSUM") as dpsum,
            tc.tile_pool(name="dmid_pool", bufs=3) as dmid_pool,
            tc.tile_pool(name="obuf_pool", bufs=4) as obuf_pool,
        ):
            # [P,1] broadcast of the cubic coefficient delivered via the
            # spilled-C3 (Src1 latch) slot of op1.
            c3t = singles.tile([P, 1], F32)
            nc.vector.memset(c3t[:], EXP_C3)

            # L/R carry 4 copies of the K=18 operand at partitions 0/32/64/96
            # so 4 matmuls can run concurrently in the PE's four 32-row
            # groups (tile_position row tiling) — hides the fp32 LDWEIGHTS
            # and 2-pass matmul cost behind concurrent streaming. The four
            # copies are independent DRAM loads spread over both DMA rings.
            L = singles.tile([3 * 32 + K, rows], F32)
            R = singles.tile([3 * 32 + K, m_cols], F32)
            for g in range(4):
                eng = nc.sync if g % 2 == 0 else nc.gpsimd
                eng.dma_start(L[32 * g : 32 * g + K, :], l18[:])
                eng.dma_start(R[32 * g : 32 * g + K, :], r18[:])

            # --- main loop ----------------------------------------------
            for _rep in range(repeats):
                for m in range(rb):
                    ob = obuf_pool.tile([P, m_cols], F16, tag="ob", name="ob")
                    for c in range(nsc):
                        base = c * SUPER
                        ps_a = apsum.tile([P, ACT_W], F32, tag="pa", name="pa")
                        ps_d = dpsum.tile([P, DVE_W], F32, tag="pd", name="pd")
                        for k in range(4):
                            col = base + k * FREE
                            dst = (ps_a[:, k * FREE : (k + 1) * FREE]
                                   if k < 3 else ps_d[:])
                            nc.tensor.matmul(
                                dst,
                                L[32 * k : 32 * k + K, m * P : (m + 1) * P],
                                R[32 * k : 32 * k + K, col : col + FREE],
                                start=True, stop=True,
                                tile_position=(32 * k, 0),
                            )
                        # ScalarE: exp on the 1536-wide chunk
                        nc.scalar.activation(
                            ob[:, base : base + ACT_W], ps_a[:],
                            AF.Exp, scale=-0.5,
                        )
                        # VectorE: fused poly exp on the 512-wide chunk
                        mid = dmid_pool.tile([P, DVE_W], F32, tag="dm",
                                             name="dm")
                        nc.vector._custom_dve(
                            op1, out=mid[:], in0=ps_d[:], in1=c3t[:],
                            s0=EXP_CLAMP, s1=EXP_C1, imm2=EXP_C2,
                        )
                        nc.vector._custom_dve(
                            op2, out=ob[:, base + ACT_W : base + SUPER],
                            in0=mid[:],
                        )
                        # store each superchunk as soon as it lands;
                        # alternate HWDGE/SWDGE rings so per-DMA completion
                        # stalls overlap across rings.
                        eng = nc.sync if (m * nsc + c) % 2 == 0 else nc.gpsimd
                        eng.dma_start(
                            out[m * P : (m + 1) * P, base : base + SUPER],
                            ob[:, base : base + SUPER],
                        )

    nc.compile()  # bacc lowering: splits multi-waits, reg alloc, etc.
    return nc


_PROGRAM_CACHE = {}


def get_program(rows=ROWS, m_cols=M, repeats=1):
    key = (rows, m_cols, repeats)
    if key not in _PROGRAM_CACHE:
        _PROGRAM_CACHE[key] = build_program(rows, m_cols, repeats)
    return _PROGRAM_CACHE[key]


def make_in_maps(U, V, weights, sn):
    """Host-side O(N*D) prep: scaled/augmented matmul operands per core."""
    U = np.asarray(U, dtype=np.float32)
    V = np.asarray(V, dtype=np.float32)
    w = np.asarray(weights, dtype=np.float32).reshape(D)
    s = np.exp(-w)                                        # (D,)
    us = U * s                                            # (N, D)
    vs = V * s                                            # (M, D)
    u2 = np.einsum("nd,nd->n", us, us, dtype=np.float32)  # (N,)
    v2 = np.einsum("md,md->m", vs, vs, dtype=np.float32)  # (M,)
    sn4 = 4.0 * float(np.asarray(sn))                     # exp(2 sn) folded in

    r18 = np.empty((K, M), dtype=np.float32)
    r18[0:D] = vs.T
    r18[D] = 1.0
    r18[D + 1] = v2
    r18 = np.ascontiguousarray(r18)

    in_maps = []
    for c in range(N_CORES):
        sl = slice(c * ROWS, (c + 1) * ROWS)
        l18 = np.empty((K, ROWS), dtype=np.float32)
        l18[0:D] = -2.0 * us[sl].T
        l18[D] = u2[sl] - sn4
        l18[D + 1] = 1.0
        in_maps.append({"l18": np.ascontiguousarray(l18), "r18": r18})
    return in_maps


def kernel(U, V, weights, sn):
    from concourse.bass_utils import run_bass_kernel_spmd

    nc = get_program()
    in_maps = make_in_maps(U, V, weights, sn)
    res = run_bass_kernel_spmd(nc, in_maps, core_ids=list(range(N_CORES)))
    return np.concatenate(
        [np.asarray(r["out"]).astype(np.float32) for r in res.results], axis=0
    )
